# revision 24
# baseline (speedup 1.0000x reference)
"""Trainium2 Bass kernel for a transformer block (LN -> causal MHA -> FFN).

Sharding (8 NeuronCores, one chip):
  - LayerNorm/RMSNorm: sequence-sharded (256 tokens/core), then chunked
    AllGather (4 chunks along d) of the transposed normed activations h^T
    (bf16) so every core holds full-seq h^T; h^T reloads pipeline per chunk.
  - Attention: head-parallel (3 of 24 heads per core, full sequence, causal,
    no-max-subtraction softmax, 1/rowsum via fast-approx reciprocal, causal
    mask applied on GpSimd).
  - o-AllToAll split in two (heads {0,1} fire mid-attention, head {2} at the
    end) converts head-sharded attention output o^T into sequence-sharded
    o^T; each core computes Wo + residual and the FFN for its own 256 tokens
    with replicated, streamed W1/W2 (prefetch starts mid-attention).
    FFN1/FFN2 interleave per 1024-f chunk with fp32 z accumulation in SBUF
    (no FFN1->FFN2 barrier).

Matmuls run in bf16 with fp32 PSUM accumulation; norms, residuals and all
reductions stay fp32.
"""

import sys

for _p in ("/opt/trn_rl_repo",):
    if _p not in sys.path:
        sys.path.append(_p)

import numpy as np
import ml_dtypes

import concourse.bass as bass
import concourse.mybir as mybir
import concourse.tile as tile
from concourse import bacc
from concourse.bass_utils import run_bass_kernel_spmd
from concourse.masks import make_identity

AF = mybir.ActivationFunctionType
ALU = mybir.AluOpType

S, D, H, Dh, F = 2048, 2048, 24, 128, 8192
N_CORES = 8
S_LOC = S // N_CORES          # 256 tokens per core
H_LOC = H // N_CORES          # 3 heads per core
CW = H_LOC * Dh               # 384 qkv columns per core
SCALE = Dh ** -0.5
EPS = 1e-5

bf16 = mybir.dt.bfloat16
f32 = mybir.dt.float32

TRACE = False        # test.py flips this for profiled runs
_CACHE = {}


def _emit(nc, tc, io):
    rg = [list(range(N_CORES))]
    x_r, lng, lnb, b2b, b1t, wqkv, wv, wo, w1, w2, msk, onc, out_r = io

    dram = tc.alloc_tile_pool(name="dram", bufs=1, space="DRAM")
    constp = tc.alloc_tile_pool(name="const", bufs=1)

    ag_in = [dram.tile([1024, S_LOC], bf16, name=f"ag_in{c}") for c in range(2)]
    ag_out = [dram.tile([N_CORES * 1024, S_LOC], bf16, addr_space="Shared",
                        name=f"ag_out{c}") for c in range(2)]
    va_in = dram.tile([N_CORES, 2 * 128 * CW], bf16)
    va_out = dram.tile([N_CORES, 2 * 128 * CW], bf16)
    a2a1_in = dram.tile([N_CORES * 2 * 128, S_LOC], bf16)
    a2a1_out = dram.tile([N_CORES * 2 * 128, S_LOC], bf16)
    a2a2_in = dram.tile([N_CORES * 128, S_LOC], bf16)
    a2a2_out = dram.tile([N_CORES * 128, S_LOC], bf16)

    # constants (DMAs issued on scalar queue so x_r goes first on sync)
    ident = constp.tile([128, 128], bf16)
    make_identity(nc, ident[:, :])
    b2r_sb = constp.tile([1, D], bf16)
    nc.scalar.dma_start(b2r_sb[:, :], b2b[:, :])
    on1_sb = constp.tile([1, 128], bf16)
    nc.scalar.dma_start(on1_sb[:, :], onc[0:1, :])
    b1t_sb = constp.tile([128, F // 128], f32)
    nc.scalar.dma_start(b1t_sb[:, :], b1t[:, :])
    msk_sb = constp.tile([128, 2048], bf16)
    nc.scalar.dma_start(msk_sb[:, :], msk[:, :])
    onc_sb = constp.tile([128, 128], bf16)
    nc.scalar.dma_start(onc_sb[:, :], onc[:, :])
    eps_sb = constp.tile([128, 1], f32)
    nc.vector.memset(eps_sb[:, :], EPS)

    # tiny dummy AllGather fired at t~0: absorbs the collective engine's
    # wakeup latency so the real AllGather chunks start without it
    warm_in = dram.tile([8, 64], bf16, name="warm_in")
    warm_out = dram.tile([64, 64], bf16, addr_space="Shared", name="warm_out")
    nc.gpsimd.dma_start(warm_in[:, :], onc[0:8, 0:64])
    nc.gpsimd.collective_compute(
        "AllGather", ALU.bypass, replica_groups=rg,
        ins=[warm_in.opt()], outs=[warm_out.opt()],
    )

    # persistent activations (whole-kernel lifetime)
    persist = tc.alloc_tile_pool(name="persist", bufs=1)
    xln = [persist.tile([128, D], bf16, name=f"xln{i}") for i in range(2)]
    z_sb = [persist.tile([128, D], f32, name=f"z{i}") for i in range(2)]

    # ---------------- Phase A: LN + RMSNorm + transpose (own tokens) -------
    sbA = tc.alloc_tile_pool(name="phA", bufs=2)
    psA = tc.alloc_tile_pool(name="phA_ps", bufs=4, space="PSUM")
    lng_sb = sbA.tile([128, D], bf16, name="lng_sb", bufs=1)
    nc.sync.dma_start(lng_sb[:, :], lng[:, :])
    lnb_sb = sbA.tile([128, D], bf16, name="lnb_sb", bufs=1)
    nc.sync.dma_start(lnb_sb[:, :], lnb[:, :])
    hT = sbA.tile([128, 16 * S_LOC], bf16, name="hT", bufs=1)
    hh_t = []
    for st in range(2):
        xa = sbA.tile([128, D], f32, tag="xa")
        nc.sync.dma_start(xa[:, :], x_r[st * 128:(st + 1) * 128, :])
        stats = sbA.tile([128, 24], f32, tag="stats")
        for a in range(4):
            nc.vector.bn_stats(stats[:, a * 6:(a + 1) * 6],
                               xa[:, a * 512:(a + 1) * 512])
        aggr = sbA.tile([128, 2], f32, tag="aggr")
        nc.vector.bn_aggr(aggr[:, :], stats[:, :].rearrange("p (a b) -> p a b", b=6))
        std = sbA.tile([128, 1], f32, tag="std")
        nc.scalar.activation(std[:, :], aggr[:, 1:2], AF.Sqrt, bias=eps_sb[:, :])
        istd = sbA.tile([128, 1], f32, tag="istd")
        nc.vector.reciprocal(istd[:, :], std[:, :])
        nc.vector.tensor_scalar(
            out=xln[st][:, :], in0=xa[:, :],
            scalar1=aggr[:, 0:1], scalar2=istd[:, :],
            op0=ALU.subtract, op1=ALU.mult,
        )
        nc.vector.tensor_tensor(xln[st][:, :], xln[st][:, :], lng_sb[:, :], op=ALU.mult)
        nc.vector.tensor_tensor(xln[st][:, :], xln[st][:, :], lnb_sb[:, :], op=ALU.add)
        # rms stats of x_ln
        stats2 = sbA.tile([128, 24], f32, tag="stats2")
        for a in range(4):
            nc.vector.bn_stats(stats2[:, a * 6:(a + 1) * 6],
                               xln[st][:, a * 512:(a + 1) * 512])
        aggr2 = sbA.tile([128, 2], f32, tag="aggr2")
        nc.vector.bn_aggr(aggr2[:, :], stats2[:, :].rearrange("p (a b) -> p a b", b=6))
        ms = sbA.tile([128, 1], f32, tag="ms")
        nc.vector.tensor_mul(ms[:, :], aggr2[:, 0:1], aggr2[:, 0:1])
        nc.vector.tensor_tensor(ms[:, :], ms[:, :], aggr2[:, 1:2], op=ALU.add)
        rstd = sbA.tile([128, 1], f32, tag="rstd")
        nc.scalar.activation(rstd[:, :], ms[:, :], AF.Sqrt, bias=eps_sb[:, :])
        irms = sbA.tile([128, 1], f32, tag="irms")
        nc.vector.reciprocal(irms[:, :], rstd[:, :])
        h = sbA.tile([128, D], bf16, tag="h")
        nc.vector.tensor_scalar(
            out=h[:, :], in0=xln[st][:, :],
            scalar1=irms[:, :], scalar2=None, op0=ALU.mult,
        )
        hh_t.append(h)
    # transposes dc-major; fire an AllGather chunk per 8-dc group
    for c in range(2):
        for dcl in range(8):
            dc = c * 8 + dcl
            for st in range(2):
                tp = psA.tile([128, 128], bf16, tag="tp")
                nc.tensor.transpose(
                    tp[:, :], hh_t[st][:, dc * 128:(dc + 1) * 128], ident[:, :])
                nc.vector.tensor_copy(
                    hT[:, dc * S_LOC + st * 128: dc * S_LOC + (st + 1) * 128],
                    tp[:, :])
        nc.gpsimd.dma_start(
            ag_in[c][:, :].rearrange("(dc p) j -> p dc j", p=128),
            hT[:, :].rearrange("p (dc j) -> p dc j", j=S_LOC)[:, 8 * c:8 * c + 8, :],
        )
        nc.gpsimd.collective_compute(
            "AllGather", ALU.bypass, replica_groups=rg,
            ins=[ag_in[c].opt()], outs=[ag_out[c].opt()],
        )
    psA.release()

    # wq for this core's 3 heads (q then k columns); linear per-partition
    wp = tc.alloc_tile_pool(name="phC_w", bufs=1, side="right")
    wq_sb = [wp.tile([128, 4 * 2 * CW], bf16, name=f"wqkv{i}") for i in range(4)]
    for g4 in range(4):
        nc.sync.dma_start(wq_sb[g4][:, :], wqkv[g4])
    wq = [wq_sb[dc // 4][:, (dc % 4) * 2 * CW:(dc % 4 + 1) * 2 * CW]
          for dc in range(16)]

    # ---------------- Phase V: v for own tokens, all heads (during AG) -----
    psV = tc.alloc_tile_pool(name="phV_ps", bufs=2, space="PSUM")
    with tc.tile_pool(name="phV_w", bufs=3) as wvp:
        for vg in range(8):
            wvg = wvp.tile([128, 16 * CW], bf16, tag="wv")
            nc.sync.dma_start(wvg[:, :], wv[vg])
            for st in range(2):
                ps = psV.tile([128, CW], f32, tag="v_ps")
                for dc in range(16):
                    nc.tensor.matmul(
                        ps[:, :],
                        lhsT=hT[:, dc * S_LOC + st * 128: dc * S_LOC + (st + 1) * 128],
                        rhs=wvg[:, dc * CW:(dc + 1) * CW],
                        start=(dc == 0), stop=(dc == 15),
                    )
                sv = sbA.tile([128, CW], bf16, tag="sv", bufs=3)
                nc.vector.tensor_copy(sv[:, :], ps[:, :])
                nc.scalar.dma_start(
                    va_in[vg, st * 128 * CW:(st + 1) * 128 * CW]
                    .rearrange("(p j) -> p j", j=CW),
                    sv[:, :],
                )
    psV.release()
    nc.gpsimd.collective_compute(
        "AllToAll", ALU.bypass, replica_groups=rg,
        ins=[va_in.opt()], outs=[va_out.opt()],
    )
    sbA.release()

    # ---------------- Phase C: QKV projections + attention -----------------
    pCD = tc.alloc_tile_pool(name="pCD", bufs=1)
    qkT = [pCD.tile([128, S], bf16, name=f"qkT{i}") for i in range(6)]
    vsb = [pCD.tile([128, CW], bf16, name=f"v{i}") for i in range(16)]
    sbD_pool = tc.alloc_tile_pool(name="phD", bufs=4)
    hp = tc.alloc_tile_pool(name="phC_h", bufs=1, side="right")
    psC = tc.alloc_tile_pool(name="phC_ps", bufs=2, space="PSUM", side="right")

    # full-seq h^T loads, pipelined per AllGather chunk
    hTb = [hp.tile([128, S], bf16, name=f"hTb{i}") for i in range(16)]
    eng = {0: nc.sync, 1: nc.scalar, 2: nc.gpsimd, 3: nc.sync}
    for dc in range(16):
        eng[(dc % 8) // 2].dma_start(
            hTb[dc][:, :].rearrange("p (r j) -> p r j", r=8),
            ag_out[dc // 8][:, :].rearrange(
                "(r q p) j -> q p r j", r=8, p=128)[dc % 8],
        )
    # v arrives via the AllToAll
    for stv in range(16):
        nc.scalar.dma_start(
            vsb[stv][:, :],
            va_out[stv // 2, (stv % 2) * 128 * CW:(stv % 2 + 1) * 128 * CW]
            .rearrange("(p j) -> p j", j=CW),
        )

    sbD = sbD_pool
    psDs = tc.alloc_tile_pool(name="phD_s", bufs=2, space="PSUM")
    psDo = tc.alloc_tile_pool(name="phD_o", bufs=1, space="PSUM")
    psDr = tc.alloc_tile_pool(name="phD_r", bufs=1, space="PSUM")

    def project_head(hh):
        for ct in (hh, 3 + hh):            # q-tile then k-tile of head hh
            for snb in range(4):
                ps = psC.tile([128, 512], f32, tag="qk_ps")
                for dc in range(16):
                    nc.tensor.matmul(
                        ps[:, :],
                        lhsT=wq[dc][:, ct * 128:(ct + 1) * 128],
                        rhs=hTb[dc][:, snb * 512:(snb + 1) * 512],
                        start=(dc == 0), stop=(dc == 15),
                    )
                nc.vector.tensor_copy(qkT[ct][:, snb * 512:(snb + 1) * 512], ps[:, :])

    def stage1(qi, hh):
        qT = qkT[hh]
        kT = qkT[3 + hh]
        npair = 2 * (qi + 1)
        p_tiles = []
        for kp in range(npair):
            s_ps = psDs.tile([128, 1024], f32, tag="s")
            for u in range(2):
                ki = 2 * kp + u
                a = ki - 4 * qi        # >0 on diagonal pairs: skip q < 128a
                qo = 128 * a if a > 0 else 0
                nc.tensor.matmul(
                    s_ps[:, u * 512 + qo:(u + 1) * 512],
                    lhsT=kT[:, ki * 128:(ki + 1) * 128],
                    rhs=qT[:, qi * 512 + qo:(qi + 1) * 512],
                    start=True, stop=True,
                )
            p_sb = sbD.tile([128, 1024], bf16, tag="p", bufs=14)
            nc.scalar.activation(p_sb[:, :], s_ps[:, :], AF.Exp, scale=SCALE)
            if kp >= 2 * qi:           # diagonal pair -> causal mask
                mh = kp - 2 * qi
                nc.gpsimd.tensor_tensor(
                    p_sb[:, :], p_sb[:, :],
                    msk_sb[:, mh * 1024:(mh + 1) * 1024], op=ALU.mult,
                )
            p_tiles.append(p_sb)
        return p_tiles

    def stage2(qi, hh, p_tiles):
        npair = 2 * (qi + 1)
        o_ps = psDo.tile([128, 512], f32, tag="o")
        r_ps = psDr.tile([128, 512], f32, tag="r")
        for kp in range(npair):
            p_sb = p_tiles[kp]
            for u in range(2):
                ki = 2 * kp + u
                a = ki - 4 * qi
                qo = 128 * a if a > 0 else 0
                nc.tensor.matmul(
                    o_ps[:, qo:512],
                    lhsT=vsb[ki][:, hh * 128:(hh + 1) * 128],
                    rhs=p_sb[:, u * 512 + qo:(u + 1) * 512],
                    start=(kp == 0 and u == 0),
                    stop=(kp == npair - 1 and u == 1),
                    skip_group_check=True,
                )
                nc.tensor.matmul(
                    r_ps[:, qo:512],
                    lhsT=onc_sb[:, :],
                    rhs=p_sb[:, u * 512 + qo:(u + 1) * 512],
                    start=(kp == 0 and u == 0),
                    stop=(kp == npair - 1 and u == 1),
                    skip_group_check=True,
                )
        rc_sb = sbD.tile([128, 512], f32, tag="rc", bufs=2)
        nc.vector.reciprocal_approx_fast(rc_sb[:, :], r_ps[:, :])
        on_sb = sbD.tile([128, 512], bf16, tag="on", bufs=2)
        nc.vector.tensor_mul(on_sb[:, :], o_ps[:, :], rc_sb[:, :])
        if hh < 2:
            nc.gpsimd.dma_start(
                a2a1_in[:, :].rearrange(
                    "(j c p) t -> c p j t", c=2, p=128)[hh][:, 2 * qi:2 * qi + 2, :],
                on_sb[:, :].rearrange("p (j t) -> p j t", j=2),
            )
        else:
            nc.gpsimd.dma_start(
                a2a2_in[:, :].rearrange(
                    "(j p) t -> p j t", p=128)[:, 2 * qi:2 * qi + 2, :],
                on_sb[:, :].rearrange("p (j t) -> p j t", j=2),
            )

    # weight pools: prefetch starts mid-attention, after hTb/wq release
    wo_t = {}
    w1t = {}
    w2t = {}
    pools = {}

    def load_wo(i):
        # i in 0..5: 0..3 = A-block tiles (16 heads), 4..5 = B-block (8 heads)
        t = pools["wo"].tile([128, 4 * D], bf16, tag="wo", name=f"wo{i}")
        nc.sync.dma_start(t[:, :], wo[i])
        wo_t[i] = t

    def load_w1(fg):
        t = pools["w1"].tile([128, 16 * 512], bf16, tag="w1", name=f"w1_{fg}")
        nc.sync.dma_start(t[:, :], w1[fg])
        w1t[fg] = t

    def load_w2(fcg, half):
        t = pools["w2"].tile([128, 4 * D], bf16, tag="w2", name=f"w2_{fcg}_{half}")
        nc.sync.dma_start(t[:, :], w2[fcg, half])
        w2t[(fcg, half)] = t

    # software pipeline across units, hh-major so each head's units
    # follow its projections immediately.
    units = [(qi, hh) for hh in range(3) for qi in range(4)]
    pending = None

    def post_unit(u):
        if u[0] == 3 and u[1] == 1:
            nc.gpsimd.collective_compute(
                "AllToAll", ALU.bypass, replica_groups=rg,
                ins=[a2a1_in.opt()], outs=[a2a1_out.opt()],
            )
        if u[0] == 3 and u[1] == 2:
            nc.gpsimd.collective_compute(
                "AllToAll", ALU.bypass, replica_groups=rg,
                ins=[a2a2_in.opt()], outs=[a2a2_out.opt()],
            )

    for (qi, hh) in units:
        if qi == 0:
            project_head(hh)
            if hh == 2:
                # hTb/wq fully consumed: free them and start weight prefetch
                hp.release()
                wp.release()
                psC.release()
                pools["wo"] = tc.alloc_tile_pool(name="wop", bufs=3, side="right")
                pools["w1"] = tc.alloc_tile_pool(name="w1p", bufs=2, side="right")
                load_wo(0)
                load_wo(1)
                load_w1(0)
                load_w1(1)
        p_tiles = stage1(qi, hh)
        if pending is not None:
            stage2(*pending[:3])
            post_unit(pending)
        pending = (qi, hh, p_tiles)
    stage2(*pending[:3])
    post_unit(pending)
    psDr.release()
    psDo.release()
    psDs.release()
    sbD_pool.release()
    pCD.release()

    # ---------------- Phase E: Wo + residual --------------------------------
    pFG = tc.alloc_tile_pool(name="pFG", bufs=1)
    yT = pFG.tile([128, 16 * S_LOC], bf16, name="yT")
    oT1 = pFG.tile([128, 16 * S_LOC], bf16, name="oT1")
    oT2 = pFG.tile([128, 8 * S_LOC], bf16, name="oT2")
    nc.gpsimd.dma_start(
        oT1[:, :].rearrange("p (g t) -> p g t", t=S_LOC),
        a2a1_out[:, :].rearrange("(g p) t -> p g t", p=128),
    )
    nc.gpsimd.dma_start(
        oT2[:, :].rearrange("p (g t) -> p g t", t=S_LOC),
        a2a2_out[:, :].rearrange("(g p) t -> p g t", p=128),
    )
    load_wo(2)
    load_wo(3)
    load_wo(4)
    load_wo(5)
    pools["w2"] = tc.alloc_tile_pool(name="w2p", bufs=2, side="right")
    load_w2(0, 0)
    load_w2(0, 1)
    psE = tc.alloc_tile_pool(name="phE_ps", bufs=8, space="PSUM")
    sbF = tc.alloc_tile_pool(name="phF", bufs=1)
    y_ps = [psE.tile([128, 512], f32, name=f"y_ps{i}", tag="y") for i in range(8)]
    for g in range(16):
        for st2 in range(2):
            for dt4 in range(4):
                nc.tensor.matmul(
                    y_ps[st2 * 4 + dt4][:, :],
                    lhsT=oT1[:, g * S_LOC + st2 * 128: g * S_LOC + (st2 + 1) * 128],
                    rhs=wo_t[g // 4][:, (g % 4) * D + dt4 * 512:
                                     (g % 4) * D + (dt4 + 1) * 512],
                    start=(g == 0), stop=False,
                )
    for g in range(8):
        for st2 in range(2):
            for dt4 in range(4):
                nc.tensor.matmul(
                    y_ps[st2 * 4 + dt4][:, :],
                    lhsT=oT2[:, g * S_LOC + st2 * 128: g * S_LOC + (st2 + 1) * 128],
                    rhs=wo_t[4 + g // 4][:, (g % 4) * D + dt4 * 512:
                                         (g % 4) * D + (dt4 + 1) * 512],
                    start=False, stop=(g == 7),
                )
    # ---- y evict (z_sb = y fp32, ybf bf16 for transpose), per d-group ------
    ybf = [sbF.tile([128, D], bf16, name=f"ybf{i}") for i in range(2)]
    for dt4 in range(4):
        for st2 in range(2):
            nc.vector.scalar_tensor_tensor(
                out=ybf[st2][:, dt4 * 512:(dt4 + 1) * 512],
                in0=y_ps[st2 * 4 + dt4][:, :], scalar=1.0,
                in1=xln[st2][:, dt4 * 512:(dt4 + 1) * 512],
                op0=ALU.mult, op1=ALU.add,
            )
    for dt4 in range(4):
        for st2 in range(2):
            nc.vector.scalar_tensor_tensor(
                out=z_sb[st2][:, dt4 * 512:(dt4 + 1) * 512],
                in0=y_ps[st2 * 4 + dt4][:, :], scalar=1.0,
                in1=xln[st2][:, dt4 * 512:(dt4 + 1) * 512],
                op0=ALU.mult, op1=ALU.add,
            )
    psE.release()
    psT = tc.alloc_tile_pool(name="phT_ps", bufs=2, space="PSUM")
    for dt4 in range(4):
        for st2 in range(2):
            tps = psT.tile([128, 512], bf16, tag="tp2")
            for dcl in range(4):
                dc = 4 * dt4 + dcl
                nc.tensor.transpose(
                    tps[:, dcl * 128:(dcl + 1) * 128],
                    ybf[st2][:, dc * 128:(dc + 1) * 128], ident[:, :])
            nc.scalar.copy(
                yT[:, :].rearrange("p (dc t) -> p dc t", t=S_LOC)
                [:, 4 * dt4:4 * dt4 + 4, st2 * 128:(st2 + 1) * 128],
                tps[:, :].rearrange("p (dc t) -> p dc t", t=128),
            )
    psT.release()

    # ---------------- Phase G/H: FFN1+FFN2 interleaved per 1024-f chunk -----
    pGH = tc.alloc_tile_pool(name="pGH", bufs=1)
    gT = pGH.tile([128, 64 * S_LOC], bf16, name="gT")
    psG = tc.alloc_tile_pool(name="phG_ps", bufs=2, space="PSUM")
    psZ = tc.alloc_tile_pool(name="phZ_ps", bufs=3, space="PSUM")

    def ffn1(fcg):
        for fgl in range(2):
            fg = 2 * fcg + fgl
            if fg + 1 <= 15 and fg + 1 not in w1t:
                load_w1(fg + 1)
            for ft in range(4):
                fi = 4 * fg + ft
                g_ps = psG.tile([128, S_LOC], f32, tag="g")
                for dc in range(16):
                    nc.tensor.matmul(
                        g_ps[:, :],
                        lhsT=w1t[fg][:, dc * 512 + ft * 128: dc * 512 + (ft + 1) * 128],
                        rhs=yT[:, dc * S_LOC:(dc + 1) * S_LOC],
                        start=(dc == 0), stop=(dc == 15),
                    )
                nc.scalar.activation(
                    gT[:, fi * S_LOC:(fi + 1) * S_LOC], g_ps[:, :],
                    AF.Gelu_apprx_tanh,
                    bias=b1t_sb[:, fi:fi + 1], scale=1.0,
                )

    def ffn2(fcg):
        if fcg + 1 <= 7:
            load_w2(fcg + 1, 0)
            load_w2(fcg + 1, 1)
        for dhalf in range(2):
            for tt in range(2):
                z_ps = psZ.tile([128, 1024], f32, tag="z")
                for dq in range(2):
                    zsl = z_ps[:, dq * 512:(dq + 1) * 512]
                    dcol = dhalf * 1024 + dq * 512
                    if fcg == 0:
                        nc.tensor.matmul(
                            zsl, lhsT=on1_sb[:, :],
                            rhs=b2r_sb[:, dcol:dcol + 512],
                            start=True, stop=False,
                        )
                    for fc in range(8):
                        fi = fcg * 8 + fc
                        nc.tensor.matmul(
                            zsl,
                            lhsT=gT[:, fi * S_LOC + tt * 128: fi * S_LOC + (tt + 1) * 128],
                            rhs=w2t[(fcg, fc // 4)][:, (fc % 4) * D + dcol:
                                                    (fc % 4) * D + dcol + 512],
                            start=(fcg != 0 and fc == 0), stop=(fc == 7),
                        )
                nc.vector.scalar_tensor_tensor(
                    out=z_sb[tt][:, dhalf * 1024:(dhalf + 1) * 1024],
                    in0=z_ps[:, :], scalar=1.0,
                    in1=z_sb[tt][:, dhalf * 1024:(dhalf + 1) * 1024],
                    op0=ALU.mult, op1=ALU.add,
                )
                if fcg == 7:
                    nc.gpsimd.dma_start(
                        out_r[tt * 128:(tt + 1) * 128, dhalf * 1024:(dhalf + 1) * 1024],
                        z_sb[tt][:, dhalf * 1024:(dhalf + 1) * 1024])

    # software pipeline: ffn2(fcg) emitted after ffn1(fcg+1) keeps PE fed
    ffn1(0)
    for fcg in range(1, 8):
        ffn1(fcg)
        ffn2(fcg - 1)
    ffn2(7)

    psZ.release()
    psG.release()
    pGH.release()
    pools["w2"].release()
    pools["w1"].release()
    pools["wo"].release()
    sbF.release()
    pFG.release()
    persist.release()
    constp.release()
    dram.release()


def _build():
    if "nc" in _CACHE:
        return _CACHE["nc"]
    nc = bacc.Bacc("TRN2", target_bir_lowering=False, debug=False,
                   num_devices=N_CORES)

    def I(name, shape, dt):
        return nc.dram_tensor(name, shape, dt, kind="ExternalInput")

    io = (
        I("x_r", [S_LOC, D], f32),
        I("lng", [128, D], bf16),
        I("lnb", [128, D], bf16),
        I("b2b", [1, D], bf16),
        I("b1t", [128, F // 128], f32),
        I("wqkv", [4, 128, 4 * 2 * CW], bf16),
        I("wv", [8, 128, 16 * CW], bf16),
        I("wo", [6, 128, 4 * D], bf16),
        I("w1", [16, 128, 16 * 512], bf16),
        I("w2", [8, 2, 128, 4 * D], bf16),
        I("msk", [128, 2048], bf16),
        I("onc", [128, 128], bf16),
        nc.dram_tensor("out_r", [S_LOC, D], f32, kind="ExternalOutput"),
    )
    with tile.TileContext(nc) as tc:
        _emit(nc, tc, io)
    nc.compile()
    _CACHE["nc"] = nc
    return nc


def _host_prep(inputs):
    bf = ml_dtypes.bfloat16
    x = np.asarray(inputs["x"], np.float32).reshape(S, D)
    ln_g = np.asarray(inputs["ln_g"], np.float32)
    ln_b = np.asarray(inputs["ln_b"], np.float32)
    attn_g = np.asarray(inputs["attn_g"], np.float32)
    Wq = np.asarray(inputs["Wq"], np.float32)
    Wk = np.asarray(inputs["Wk"], np.float32)
    Wv = np.asarray(inputs["Wv"], np.float32)
    Wo = np.asarray(inputs["Wo"], np.float32)
    W1 = np.asarray(inputs["W1"], np.float32)
    b1 = np.asarray(inputs["b1"], np.float32)
    W2 = np.asarray(inputs["W2"], np.float32)
    b2 = np.asarray(inputs["b2"], np.float32)

    g = attn_g[:, None]
    Wq_s = (Wq * g).astype(bf)
    Wk_s = (Wk * g).astype(bf)
    Wv_s = (Wv * g).astype(bf)
    # wo reordered: A-block heads (3s, 3s+1), then B-block heads (3s+2);
    # then [6, 128, 4*D] with per-partition-contiguous lines
    heads = [3 * s + c for s in range(N_CORES) for c in range(2)] + \
            [3 * s + 2 for s in range(N_CORES)]
    wo_b = np.ascontiguousarray(
        Wo.reshape(H, Dh, D)[heads].reshape(6, 4, 128, D)
        .transpose(0, 2, 1, 3).reshape(6, 128, 4 * D)).astype(bf)
    # w1_b[fg, p, dc*512+j] = W1[dc*128+p, fg*512+j]
    w1_b = np.ascontiguousarray(
        W1.reshape(16, 128, 16, 512).transpose(2, 1, 0, 3)
        .reshape(16, 128, 16 * 512)).astype(bf)
    # w2_b[fcg, half, p, fc*D+d] = W2[fcg*1024+half*512+fc*128+p, d]
    w2_b = np.ascontiguousarray(
        W2.reshape(8, 2, 4, 128, D).transpose(0, 1, 3, 2, 4)
        .reshape(8, 2, 128, 4 * D)).astype(bf)
    # wv_blk[vg, p, dc*CW+j] = Wv[dc*128+p, vg*CW+j]
    wv_blk = np.ascontiguousarray(
        Wv_s.reshape(16, 128, N_CORES, CW).transpose(2, 1, 0, 3)
        .reshape(N_CORES, 128, 16 * CW))

    lng_b = np.broadcast_to(ln_g[None, :], (128, D)).astype(bf)
    lnb_b = np.broadcast_to(ln_b[None, :], (128, D)).astype(bf)
    b2_b = b2[None, :].astype(bf)
    b1_t = np.ascontiguousarray(b1.reshape(F // 128, 128).T).astype(np.float32)

    i_idx = np.arange(512)[None, :]
    j_idx = np.arange(128)[:, None]
    msk = np.concatenate(
        [(i_idx >= 128 * m + j_idx) for m in range(4)], axis=1
    ).astype(bf)
    onc = np.ones((128, 128), bf)

    in_maps = []
    for r in range(N_CORES):
        wqkv_r = np.concatenate(
            [Wq_s[:, r * CW:(r + 1) * CW],
             Wk_s[:, r * CW:(r + 1) * CW]], axis=1)
        wqkv_r = np.ascontiguousarray(
            wqkv_r.reshape(4, 4, 128, 2 * CW).transpose(0, 2, 1, 3)
            .reshape(4, 128, 4 * 2 * CW))
        in_maps.append({
            "x_r": np.ascontiguousarray(x[r * S_LOC:(r + 1) * S_LOC, :]),
            "lng": lng_b, "lnb": lnb_b, "b2b": b2_b, "b1t": b1_t,
            "wqkv": np.ascontiguousarray(wqkv_r),
            "wv": wv_blk,
            "wo": wo_b, "w1": w1_b, "w2": w2_b,
            "msk": msk, "onc": onc,
        })
    return in_maps


def kernel(**inputs) -> np.ndarray:
    nc = _build()
    in_maps = _host_prep(inputs)
    res = run_bass_kernel_spmd(
        nc, in_maps, core_ids=list(range(N_CORES)), trace=TRACE)
    _CACHE["last_result"] = res
    out = np.concatenate([res.results[r]["out_r"] for r in range(N_CORES)], axis=0)
    return out.reshape(1, S, D)


# revision 25
# speedup vs baseline: 1.0060x; 1.0060x over previous
"""Trainium2 Bass kernel for a transformer block (LN -> causal MHA -> FFN).

Sharding (8 NeuronCores, one chip):
  - LayerNorm/RMSNorm: sequence-sharded (256 tokens/core), then a 2-chunk
    AllGather of the transposed normed activations h^T (bf16) so every core
    holds full-seq h^T; the h^T reloads are pipelined per chunk with their
    DMA descriptor generation spread over three engines.
  - Attention: head-parallel (3 of 24 heads per core, full sequence, causal
    with 128-granular trimming of the diagonal blocks, no-max-subtraction
    softmax, 1/rowsum via fast-approx reciprocal on DVE, causal mask applied
    on GpSimd, exp on the scalar engine).
  - o-AllToAll split in two (heads {0,1} fire mid-attention, head {2} at the
    end) converts head-sharded attention output o^T into sequence-sharded
    o^T; each core computes Wo + residual and the FFN for its own 256 tokens
    with replicated, streamed W1/W2 (prefetch starts mid-attention, right
    after the h^T/Wqk buffers are released). FFN1/FFN2 interleave per
    1024-f chunk with fp32 z accumulation in SBUF (no FFN1->FFN2 barrier).
    All weight layouts are pre-blocked host-side so every weight DMA is a
    contiguous per-partition stream.

Matmuls run in bf16 with fp32 PSUM accumulation; norms, residuals and all
reductions stay fp32.
"""

import sys

for _p in ("/opt/trn_rl_repo",):
    if _p not in sys.path:
        sys.path.append(_p)

import numpy as np
import ml_dtypes

import concourse.bass as bass
import concourse.mybir as mybir
import concourse.tile as tile
from concourse import bacc
from concourse.bass_utils import run_bass_kernel_spmd
from concourse.masks import make_identity

AF = mybir.ActivationFunctionType
ALU = mybir.AluOpType

S, D, H, Dh, F = 2048, 2048, 24, 128, 8192
N_CORES = 8
S_LOC = S // N_CORES          # 256 tokens per core
H_LOC = H // N_CORES          # 3 heads per core
CW = H_LOC * Dh               # 384 qkv columns per core
SCALE = Dh ** -0.5
EPS = 1e-5

bf16 = mybir.dt.bfloat16
f32 = mybir.dt.float32

TRACE = False        # test.py flips this for profiled runs
_CACHE = {}


def _emit(nc, tc, io):
    rg = [list(range(N_CORES))]
    x_r, lng, lnb, b2b, b1t, wqkv, wv, wo, w1, w2, msk, onc, out_r = io

    dram = tc.alloc_tile_pool(name="dram", bufs=1, space="DRAM")
    constp = tc.alloc_tile_pool(name="const", bufs=1)

    ag_in = [dram.tile([1024, S_LOC], bf16, name=f"ag_in{c}") for c in range(2)]
    ag_out = [dram.tile([N_CORES * 1024, S_LOC], bf16, addr_space="Shared",
                        name=f"ag_out{c}") for c in range(2)]
    va_in = dram.tile([N_CORES, 2 * 128 * CW], bf16)
    va_out = dram.tile([N_CORES, 2 * 128 * CW], bf16)
    a2a1_in = dram.tile([N_CORES * 2 * 128, S_LOC], bf16)
    a2a1_out = dram.tile([N_CORES * 2 * 128, S_LOC], bf16)
    a2a2_in = dram.tile([N_CORES * 128, S_LOC], bf16)
    a2a2_out = dram.tile([N_CORES * 128, S_LOC], bf16)

    # constants (DMAs issued on scalar queue so x_r goes first on sync)
    ident = constp.tile([128, 128], bf16)
    make_identity(nc, ident[:, :])
    b2r_sb = constp.tile([1, D], bf16)
    nc.scalar.dma_start(b2r_sb[:, :], b2b[:, :])
    on1_sb = constp.tile([1, 128], bf16)
    nc.scalar.dma_start(on1_sb[:, :], onc[0:1, :])
    b1t_sb = constp.tile([128, F // 128], f32)
    nc.scalar.dma_start(b1t_sb[:, :], b1t[:, :])
    msk_sb = constp.tile([128, 2048], bf16)
    nc.scalar.dma_start(msk_sb[:, :], msk[:, :])
    onc_sb = constp.tile([128, 128], bf16)
    nc.scalar.dma_start(onc_sb[:, :], onc[:, :])
    eps_sb = constp.tile([128, 1], f32)
    nc.vector.memset(eps_sb[:, :], EPS)

    # persistent activations (whole-kernel lifetime)
    persist = tc.alloc_tile_pool(name="persist", bufs=1)
    xln = [persist.tile([128, D], bf16, name=f"xln{i}") for i in range(2)]
    z_sb = [persist.tile([128, D], f32, name=f"z{i}") for i in range(2)]

    # ---------------- Phase A: LN + RMSNorm + transpose (own tokens) -------
    sbA = tc.alloc_tile_pool(name="phA", bufs=2)
    psA = tc.alloc_tile_pool(name="phA_ps", bufs=4, space="PSUM")
    lng_sb = sbA.tile([128, D], bf16, name="lng_sb", bufs=1)
    nc.sync.dma_start(lng_sb[:, :], lng[:, :])
    lnb_sb = sbA.tile([128, D], bf16, name="lnb_sb", bufs=1)
    nc.sync.dma_start(lnb_sb[:, :], lnb[:, :])
    hT = sbA.tile([128, 16 * S_LOC], bf16, name="hT", bufs=1)
    hh_t = []
    for st in range(2):
        xa = sbA.tile([128, D], f32, tag="xa")
        nc.sync.dma_start(xa[:, :], x_r[st * 128:(st + 1) * 128, :])
        stats = sbA.tile([128, 24], f32, tag="stats")
        for a in range(4):
            nc.vector.bn_stats(stats[:, a * 6:(a + 1) * 6],
                               xa[:, a * 512:(a + 1) * 512])
        aggr = sbA.tile([128, 2], f32, tag="aggr")
        nc.vector.bn_aggr(aggr[:, :], stats[:, :].rearrange("p (a b) -> p a b", b=6))
        std = sbA.tile([128, 1], f32, tag="std")
        nc.scalar.activation(std[:, :], aggr[:, 1:2], AF.Sqrt, bias=eps_sb[:, :])
        istd = sbA.tile([128, 1], f32, tag="istd")
        nc.vector.reciprocal(istd[:, :], std[:, :])
        nc.vector.tensor_scalar(
            out=xln[st][:, :], in0=xa[:, :],
            scalar1=aggr[:, 0:1], scalar2=istd[:, :],
            op0=ALU.subtract, op1=ALU.mult,
        )
        nc.vector.tensor_tensor(xln[st][:, :], xln[st][:, :], lng_sb[:, :], op=ALU.mult)
        nc.vector.tensor_tensor(xln[st][:, :], xln[st][:, :], lnb_sb[:, :], op=ALU.add)
        # rms stats of x_ln
        stats2 = sbA.tile([128, 24], f32, tag="stats2")
        for a in range(4):
            nc.vector.bn_stats(stats2[:, a * 6:(a + 1) * 6],
                               xln[st][:, a * 512:(a + 1) * 512])
        aggr2 = sbA.tile([128, 2], f32, tag="aggr2")
        nc.vector.bn_aggr(aggr2[:, :], stats2[:, :].rearrange("p (a b) -> p a b", b=6))
        ms = sbA.tile([128, 1], f32, tag="ms")
        nc.vector.tensor_mul(ms[:, :], aggr2[:, 0:1], aggr2[:, 0:1])
        nc.vector.tensor_tensor(ms[:, :], ms[:, :], aggr2[:, 1:2], op=ALU.add)
        rstd = sbA.tile([128, 1], f32, tag="rstd")
        nc.scalar.activation(rstd[:, :], ms[:, :], AF.Sqrt, bias=eps_sb[:, :])
        irms = sbA.tile([128, 1], f32, tag="irms")
        nc.vector.reciprocal(irms[:, :], rstd[:, :])
        h = sbA.tile([128, D], bf16, tag="h")
        nc.vector.tensor_scalar(
            out=h[:, :], in0=xln[st][:, :],
            scalar1=irms[:, :], scalar2=None, op0=ALU.mult,
        )
        hh_t.append(h)
    # transposes dc-major; fire an AllGather chunk per 8-dc group
    for c in range(2):
        for dcl in range(8):
            dc = c * 8 + dcl
            for st in range(2):
                tp = psA.tile([128, 128], bf16, tag="tp")
                nc.tensor.transpose(
                    tp[:, :], hh_t[st][:, dc * 128:(dc + 1) * 128], ident[:, :])
                nc.vector.tensor_copy(
                    hT[:, dc * S_LOC + st * 128: dc * S_LOC + (st + 1) * 128],
                    tp[:, :])
        nc.gpsimd.dma_start(
            ag_in[c][:, :].rearrange("(dc p) j -> p dc j", p=128),
            hT[:, :].rearrange("p (dc j) -> p dc j", j=S_LOC)[:, 8 * c:8 * c + 8, :],
        )
        nc.gpsimd.collective_compute(
            "AllGather", ALU.bypass, replica_groups=rg,
            ins=[ag_in[c].opt()], outs=[ag_out[c].opt()],
        )
    psA.release()

    # wq for this core's 3 heads (q then k columns); linear per-partition
    wp = tc.alloc_tile_pool(name="phC_w", bufs=1, side="right")
    wq_sb = [wp.tile([128, 4 * 2 * CW], bf16, name=f"wqkv{i}") for i in range(4)]
    for g4 in range(4):
        nc.sync.dma_start(wq_sb[g4][:, :], wqkv[g4])
    wq = [wq_sb[dc // 4][:, (dc % 4) * 2 * CW:(dc % 4 + 1) * 2 * CW]
          for dc in range(16)]

    # ---------------- Phase V: v for own tokens, all heads (during AG) -----
    psV = tc.alloc_tile_pool(name="phV_ps", bufs=2, space="PSUM")
    with tc.tile_pool(name="phV_w", bufs=3) as wvp:
        for vg in range(8):
            wvg = wvp.tile([128, 16 * CW], bf16, tag="wv")
            nc.sync.dma_start(wvg[:, :], wv[vg])
            for st in range(2):
                ps = psV.tile([128, CW], f32, tag="v_ps")
                for dc in range(16):
                    nc.tensor.matmul(
                        ps[:, :],
                        lhsT=hT[:, dc * S_LOC + st * 128: dc * S_LOC + (st + 1) * 128],
                        rhs=wvg[:, dc * CW:(dc + 1) * CW],
                        start=(dc == 0), stop=(dc == 15),
                    )
                sv = sbA.tile([128, CW], bf16, tag="sv", bufs=3)
                nc.vector.tensor_copy(sv[:, :], ps[:, :])
                nc.scalar.dma_start(
                    va_in[vg, st * 128 * CW:(st + 1) * 128 * CW]
                    .rearrange("(p j) -> p j", j=CW),
                    sv[:, :],
                )
    psV.release()
    nc.gpsimd.collective_compute(
        "AllToAll", ALU.bypass, replica_groups=rg,
        ins=[va_in.opt()], outs=[va_out.opt()],
    )
    sbA.release()

    # ---------------- Phase C: QKV projections + attention -----------------
    pCD = tc.alloc_tile_pool(name="pCD", bufs=1)
    qkT = [pCD.tile([128, S], bf16, name=f"qkT{i}") for i in range(6)]
    vsb = [pCD.tile([128, CW], bf16, name=f"v{i}") for i in range(16)]
    sbD_pool = tc.alloc_tile_pool(name="phD", bufs=4)
    hp = tc.alloc_tile_pool(name="phC_h", bufs=1, side="right")
    psC = tc.alloc_tile_pool(name="phC_ps", bufs=2, space="PSUM", side="right")

    # full-seq h^T loads, pipelined per AllGather chunk
    hTb = [hp.tile([128, S], bf16, name=f"hTb{i}") for i in range(16)]
    eng = {0: nc.sync, 1: nc.scalar, 2: nc.gpsimd, 3: nc.sync}
    for dc in range(16):
        eng[(dc % 8) // 2].dma_start(
            hTb[dc][:, :].rearrange("p (r j) -> p r j", r=8),
            ag_out[dc // 8][:, :].rearrange(
                "(r q p) j -> q p r j", r=8, p=128)[dc % 8],
        )
    # v arrives via the AllToAll
    for stv in range(16):
        nc.scalar.dma_start(
            vsb[stv][:, :],
            va_out[stv // 2, (stv % 2) * 128 * CW:(stv % 2 + 1) * 128 * CW]
            .rearrange("(p j) -> p j", j=CW),
        )

    sbD = sbD_pool
    psDs = tc.alloc_tile_pool(name="phD_s", bufs=2, space="PSUM")
    psDo = tc.alloc_tile_pool(name="phD_o", bufs=1, space="PSUM")
    psDr = tc.alloc_tile_pool(name="phD_r", bufs=1, space="PSUM")

    def project_head(hh):
        for ct in (hh, 3 + hh):            # q-tile then k-tile of head hh
            for snb in range(4):
                ps = psC.tile([128, 512], f32, tag="qk_ps")
                for dc in range(16):
                    nc.tensor.matmul(
                        ps[:, :],
                        lhsT=wq[dc][:, ct * 128:(ct + 1) * 128],
                        rhs=hTb[dc][:, snb * 512:(snb + 1) * 512],
                        start=(dc == 0), stop=(dc == 15),
                    )
                nc.vector.tensor_copy(qkT[ct][:, snb * 512:(snb + 1) * 512], ps[:, :])

    def stage1(qi, hh):
        qT = qkT[hh]
        kT = qkT[3 + hh]
        npair = 2 * (qi + 1)
        p_tiles = []
        for kp in range(npair):
            s_ps = psDs.tile([128, 1024], f32, tag="s")
            for u in range(2):
                ki = 2 * kp + u
                a = ki - 4 * qi        # >0 on diagonal pairs: skip q < 128a
                qo = 128 * a if a > 0 else 0
                nc.tensor.matmul(
                    s_ps[:, u * 512 + qo:(u + 1) * 512],
                    lhsT=kT[:, ki * 128:(ki + 1) * 128],
                    rhs=qT[:, qi * 512 + qo:(qi + 1) * 512],
                    start=True, stop=True,
                )
            p_sb = sbD.tile([128, 1024], bf16, tag="p", bufs=14)
            nc.scalar.activation(p_sb[:, :], s_ps[:, :], AF.Exp, scale=SCALE)
            if kp >= 2 * qi:           # diagonal pair -> causal mask
                mh = kp - 2 * qi
                nc.gpsimd.tensor_tensor(
                    p_sb[:, :], p_sb[:, :],
                    msk_sb[:, mh * 1024:(mh + 1) * 1024], op=ALU.mult,
                )
            p_tiles.append(p_sb)
        return p_tiles

    def stage2(qi, hh, p_tiles):
        npair = 2 * (qi + 1)
        o_ps = psDo.tile([128, 512], f32, tag="o")
        r_ps = psDr.tile([128, 512], f32, tag="r")
        for kp in range(npair):
            p_sb = p_tiles[kp]
            for u in range(2):
                ki = 2 * kp + u
                a = ki - 4 * qi
                qo = 128 * a if a > 0 else 0
                nc.tensor.matmul(
                    o_ps[:, qo:512],
                    lhsT=vsb[ki][:, hh * 128:(hh + 1) * 128],
                    rhs=p_sb[:, u * 512 + qo:(u + 1) * 512],
                    start=(kp == 0 and u == 0),
                    stop=(kp == npair - 1 and u == 1),
                    skip_group_check=True,
                )
                nc.tensor.matmul(
                    r_ps[:, qo:512],
                    lhsT=onc_sb[:, :],
                    rhs=p_sb[:, u * 512 + qo:(u + 1) * 512],
                    start=(kp == 0 and u == 0),
                    stop=(kp == npair - 1 and u == 1),
                    skip_group_check=True,
                )
        rc_sb = sbD.tile([128, 512], f32, tag="rc", bufs=2)
        nc.vector.reciprocal_approx_fast(rc_sb[:, :], r_ps[:, :])
        on_sb = sbD.tile([128, 512], bf16, tag="on", bufs=2)
        nc.vector.tensor_mul(on_sb[:, :], o_ps[:, :], rc_sb[:, :])
        if hh < 2:
            nc.gpsimd.dma_start(
                a2a1_in[:, :].rearrange(
                    "(j c p) t -> c p j t", c=2, p=128)[hh][:, 2 * qi:2 * qi + 2, :],
                on_sb[:, :].rearrange("p (j t) -> p j t", j=2),
            )
        else:
            nc.gpsimd.dma_start(
                a2a2_in[:, :].rearrange(
                    "(j p) t -> p j t", p=128)[:, 2 * qi:2 * qi + 2, :],
                on_sb[:, :].rearrange("p (j t) -> p j t", j=2),
            )

    # weight pools: prefetch starts mid-attention, after hTb/wq release
    wo_t = {}
    w1t = {}
    w2t = {}
    pools = {}

    def load_wo(i):
        # i in 0..5: 0..3 = A-block tiles (16 heads), 4..5 = B-block (8 heads)
        t = pools["wo"].tile([128, 4 * D], bf16, tag="wo", name=f"wo{i}")
        nc.sync.dma_start(t[:, :], wo[i])
        wo_t[i] = t

    def load_w1(fg):
        t = pools["w1"].tile([128, 16 * 512], bf16, tag="w1", name=f"w1_{fg}")
        nc.sync.dma_start(t[:, :], w1[fg])
        w1t[fg] = t

    def load_w2(fcg, half):
        t = pools["w2"].tile([128, 4 * D], bf16, tag="w2", name=f"w2_{fcg}_{half}")
        nc.sync.dma_start(t[:, :], w2[fcg, half])
        w2t[(fcg, half)] = t

    # software pipeline across units, hh-major so each head's units
    # follow its projections immediately.
    units = [(qi, hh) for hh in range(3) for qi in range(4)]
    pending = None

    def post_unit(u):
        if u[0] == 3 and u[1] == 1:
            nc.gpsimd.collective_compute(
                "AllToAll", ALU.bypass, replica_groups=rg,
                ins=[a2a1_in.opt()], outs=[a2a1_out.opt()],
            )
        if u[0] == 3 and u[1] == 2:
            nc.gpsimd.collective_compute(
                "AllToAll", ALU.bypass, replica_groups=rg,
                ins=[a2a2_in.opt()], outs=[a2a2_out.opt()],
            )

    for (qi, hh) in units:
        if qi == 0:
            project_head(hh)
            if hh == 2:
                # hTb/wq fully consumed: free them and start weight prefetch
                hp.release()
                wp.release()
                psC.release()
                pools["wo"] = tc.alloc_tile_pool(name="wop", bufs=3, side="right")
                pools["w1"] = tc.alloc_tile_pool(name="w1p", bufs=2, side="right")
                load_wo(0)
                load_wo(1)
                load_w1(0)
                load_w1(1)
        p_tiles = stage1(qi, hh)
        if pending is not None:
            stage2(*pending[:3])
            post_unit(pending)
        pending = (qi, hh, p_tiles)
    stage2(*pending[:3])
    post_unit(pending)
    psDr.release()
    psDo.release()
    psDs.release()
    sbD_pool.release()
    pCD.release()

    # ---------------- Phase E: Wo + residual --------------------------------
    pFG = tc.alloc_tile_pool(name="pFG", bufs=1)
    yT = pFG.tile([128, 16 * S_LOC], bf16, name="yT")
    oT1 = pFG.tile([128, 16 * S_LOC], bf16, name="oT1")
    oT2 = pFG.tile([128, 8 * S_LOC], bf16, name="oT2")
    nc.gpsimd.dma_start(
        oT1[:, :].rearrange("p (g t) -> p g t", t=S_LOC),
        a2a1_out[:, :].rearrange("(g p) t -> p g t", p=128),
    )
    nc.gpsimd.dma_start(
        oT2[:, :].rearrange("p (g t) -> p g t", t=S_LOC),
        a2a2_out[:, :].rearrange("(g p) t -> p g t", p=128),
    )
    load_wo(2)
    load_wo(3)
    load_wo(4)
    load_wo(5)
    pools["w2"] = tc.alloc_tile_pool(name="w2p", bufs=2, side="right")
    load_w2(0, 0)
    load_w2(0, 1)
    psE = tc.alloc_tile_pool(name="phE_ps", bufs=8, space="PSUM")
    sbF = tc.alloc_tile_pool(name="phF", bufs=1)
    y_ps = [psE.tile([128, 512], f32, name=f"y_ps{i}", tag="y") for i in range(8)]
    for g in range(16):
        for st2 in range(2):
            for dt4 in range(4):
                nc.tensor.matmul(
                    y_ps[st2 * 4 + dt4][:, :],
                    lhsT=oT1[:, g * S_LOC + st2 * 128: g * S_LOC + (st2 + 1) * 128],
                    rhs=wo_t[g // 4][:, (g % 4) * D + dt4 * 512:
                                     (g % 4) * D + (dt4 + 1) * 512],
                    start=(g == 0), stop=False,
                )
    for g in range(8):
        for st2 in range(2):
            for dt4 in range(4):
                nc.tensor.matmul(
                    y_ps[st2 * 4 + dt4][:, :],
                    lhsT=oT2[:, g * S_LOC + st2 * 128: g * S_LOC + (st2 + 1) * 128],
                    rhs=wo_t[4 + g // 4][:, (g % 4) * D + dt4 * 512:
                                         (g % 4) * D + (dt4 + 1) * 512],
                    start=False, stop=(g == 7),
                )
    # ---- y evict (z_sb = y fp32, ybf bf16 for transpose), per d-group ------
    ybf = [sbF.tile([128, D], bf16, name=f"ybf{i}") for i in range(2)]
    for dt4 in range(4):
        for st2 in range(2):
            nc.vector.scalar_tensor_tensor(
                out=ybf[st2][:, dt4 * 512:(dt4 + 1) * 512],
                in0=y_ps[st2 * 4 + dt4][:, :], scalar=1.0,
                in1=xln[st2][:, dt4 * 512:(dt4 + 1) * 512],
                op0=ALU.mult, op1=ALU.add,
            )
    for dt4 in range(4):
        for st2 in range(2):
            nc.vector.scalar_tensor_tensor(
                out=z_sb[st2][:, dt4 * 512:(dt4 + 1) * 512],
                in0=y_ps[st2 * 4 + dt4][:, :], scalar=1.0,
                in1=xln[st2][:, dt4 * 512:(dt4 + 1) * 512],
                op0=ALU.mult, op1=ALU.add,
            )
    psE.release()
    psT = tc.alloc_tile_pool(name="phT_ps", bufs=2, space="PSUM")
    for dt4 in range(4):
        for st2 in range(2):
            tps = psT.tile([128, 512], bf16, tag="tp2")
            for dcl in range(4):
                dc = 4 * dt4 + dcl
                nc.tensor.transpose(
                    tps[:, dcl * 128:(dcl + 1) * 128],
                    ybf[st2][:, dc * 128:(dc + 1) * 128], ident[:, :])
            nc.scalar.copy(
                yT[:, :].rearrange("p (dc t) -> p dc t", t=S_LOC)
                [:, 4 * dt4:4 * dt4 + 4, st2 * 128:(st2 + 1) * 128],
                tps[:, :].rearrange("p (dc t) -> p dc t", t=128),
            )
    psT.release()

    # ---------------- Phase G/H: FFN1+FFN2 interleaved per 1024-f chunk -----
    pGH = tc.alloc_tile_pool(name="pGH", bufs=1)
    gT = pGH.tile([128, 64 * S_LOC], bf16, name="gT")
    psG = tc.alloc_tile_pool(name="phG_ps", bufs=2, space="PSUM")
    psZ = tc.alloc_tile_pool(name="phZ_ps", bufs=3, space="PSUM")

    def ffn1(fcg):
        for fgl in range(2):
            fg = 2 * fcg + fgl
            if fg + 1 <= 15 and fg + 1 not in w1t:
                load_w1(fg + 1)
            for ft in range(4):
                fi = 4 * fg + ft
                g_ps = psG.tile([128, S_LOC], f32, tag="g")
                for dc in range(16):
                    nc.tensor.matmul(
                        g_ps[:, :],
                        lhsT=w1t[fg][:, dc * 512 + ft * 128: dc * 512 + (ft + 1) * 128],
                        rhs=yT[:, dc * S_LOC:(dc + 1) * S_LOC],
                        start=(dc == 0), stop=(dc == 15),
                    )
                nc.scalar.activation(
                    gT[:, fi * S_LOC:(fi + 1) * S_LOC], g_ps[:, :],
                    AF.Gelu_apprx_tanh,
                    bias=b1t_sb[:, fi:fi + 1], scale=1.0,
                )

    def ffn2(fcg):
        if fcg + 1 <= 7:
            load_w2(fcg + 1, 0)
            load_w2(fcg + 1, 1)
        for dhalf in range(2):
            for tt in range(2):
                z_ps = psZ.tile([128, 1024], f32, tag="z")
                for dq in range(2):
                    zsl = z_ps[:, dq * 512:(dq + 1) * 512]
                    dcol = dhalf * 1024 + dq * 512
                    if fcg == 0:
                        nc.tensor.matmul(
                            zsl, lhsT=on1_sb[:, :],
                            rhs=b2r_sb[:, dcol:dcol + 512],
                            start=True, stop=False,
                        )
                    for fc in range(8):
                        fi = fcg * 8 + fc
                        nc.tensor.matmul(
                            zsl,
                            lhsT=gT[:, fi * S_LOC + tt * 128: fi * S_LOC + (tt + 1) * 128],
                            rhs=w2t[(fcg, fc // 4)][:, (fc % 4) * D + dcol:
                                                    (fc % 4) * D + dcol + 512],
                            start=(fcg != 0 and fc == 0), stop=(fc == 7),
                        )
                nc.vector.scalar_tensor_tensor(
                    out=z_sb[tt][:, dhalf * 1024:(dhalf + 1) * 1024],
                    in0=z_ps[:, :], scalar=1.0,
                    in1=z_sb[tt][:, dhalf * 1024:(dhalf + 1) * 1024],
                    op0=ALU.mult, op1=ALU.add,
                )
                if fcg == 7:
                    nc.gpsimd.dma_start(
                        out_r[tt * 128:(tt + 1) * 128, dhalf * 1024:(dhalf + 1) * 1024],
                        z_sb[tt][:, dhalf * 1024:(dhalf + 1) * 1024])

    # software pipeline: ffn2(fcg) emitted after ffn1(fcg+1) keeps PE fed
    ffn1(0)
    for fcg in range(1, 8):
        ffn1(fcg)
        ffn2(fcg - 1)
    ffn2(7)

    psZ.release()
    psG.release()
    pGH.release()
    pools["w2"].release()
    pools["w1"].release()
    pools["wo"].release()
    sbF.release()
    pFG.release()
    persist.release()
    constp.release()
    dram.release()


def _build():
    if "nc" in _CACHE:
        return _CACHE["nc"]
    nc = bacc.Bacc("TRN2", target_bir_lowering=False, debug=False,
                   num_devices=N_CORES)

    def I(name, shape, dt):
        return nc.dram_tensor(name, shape, dt, kind="ExternalInput")

    io = (
        I("x_r", [S_LOC, D], f32),
        I("lng", [128, D], bf16),
        I("lnb", [128, D], bf16),
        I("b2b", [1, D], bf16),
        I("b1t", [128, F // 128], f32),
        I("wqkv", [4, 128, 4 * 2 * CW], bf16),
        I("wv", [8, 128, 16 * CW], bf16),
        I("wo", [6, 128, 4 * D], bf16),
        I("w1", [16, 128, 16 * 512], bf16),
        I("w2", [8, 2, 128, 4 * D], bf16),
        I("msk", [128, 2048], bf16),
        I("onc", [128, 128], bf16),
        nc.dram_tensor("out_r", [S_LOC, D], f32, kind="ExternalOutput"),
    )
    with tile.TileContext(nc) as tc:
        _emit(nc, tc, io)
    nc.compile()
    _CACHE["nc"] = nc
    return nc


def _host_prep(inputs):
    bf = ml_dtypes.bfloat16
    x = np.asarray(inputs["x"], np.float32).reshape(S, D)
    ln_g = np.asarray(inputs["ln_g"], np.float32)
    ln_b = np.asarray(inputs["ln_b"], np.float32)
    attn_g = np.asarray(inputs["attn_g"], np.float32)
    Wq = np.asarray(inputs["Wq"], np.float32)
    Wk = np.asarray(inputs["Wk"], np.float32)
    Wv = np.asarray(inputs["Wv"], np.float32)
    Wo = np.asarray(inputs["Wo"], np.float32)
    W1 = np.asarray(inputs["W1"], np.float32)
    b1 = np.asarray(inputs["b1"], np.float32)
    W2 = np.asarray(inputs["W2"], np.float32)
    b2 = np.asarray(inputs["b2"], np.float32)

    g = attn_g[:, None]
    Wq_s = (Wq * g).astype(bf)
    Wk_s = (Wk * g).astype(bf)
    Wv_s = (Wv * g).astype(bf)
    # wo reordered: A-block heads (3s, 3s+1), then B-block heads (3s+2);
    # then [6, 128, 4*D] with per-partition-contiguous lines
    heads = [3 * s + c for s in range(N_CORES) for c in range(2)] + \
            [3 * s + 2 for s in range(N_CORES)]
    wo_b = np.ascontiguousarray(
        Wo.reshape(H, Dh, D)[heads].reshape(6, 4, 128, D)
        .transpose(0, 2, 1, 3).reshape(6, 128, 4 * D)).astype(bf)
    # w1_b[fg, p, dc*512+j] = W1[dc*128+p, fg*512+j]
    w1_b = np.ascontiguousarray(
        W1.reshape(16, 128, 16, 512).transpose(2, 1, 0, 3)
        .reshape(16, 128, 16 * 512)).astype(bf)
    # w2_b[fcg, half, p, fc*D+d] = W2[fcg*1024+half*512+fc*128+p, d]
    w2_b = np.ascontiguousarray(
        W2.reshape(8, 2, 4, 128, D).transpose(0, 1, 3, 2, 4)
        .reshape(8, 2, 128, 4 * D)).astype(bf)
    # wv_blk[vg, p, dc*CW+j] = Wv[dc*128+p, vg*CW+j]
    wv_blk = np.ascontiguousarray(
        Wv_s.reshape(16, 128, N_CORES, CW).transpose(2, 1, 0, 3)
        .reshape(N_CORES, 128, 16 * CW))

    lng_b = np.broadcast_to(ln_g[None, :], (128, D)).astype(bf)
    lnb_b = np.broadcast_to(ln_b[None, :], (128, D)).astype(bf)
    b2_b = b2[None, :].astype(bf)
    b1_t = np.ascontiguousarray(b1.reshape(F // 128, 128).T).astype(np.float32)

    i_idx = np.arange(512)[None, :]
    j_idx = np.arange(128)[:, None]
    msk = np.concatenate(
        [(i_idx >= 128 * m + j_idx) for m in range(4)], axis=1
    ).astype(bf)
    onc = np.ones((128, 128), bf)

    in_maps = []
    for r in range(N_CORES):
        wqkv_r = np.concatenate(
            [Wq_s[:, r * CW:(r + 1) * CW],
             Wk_s[:, r * CW:(r + 1) * CW]], axis=1)
        wqkv_r = np.ascontiguousarray(
            wqkv_r.reshape(4, 4, 128, 2 * CW).transpose(0, 2, 1, 3)
            .reshape(4, 128, 4 * 2 * CW))
        in_maps.append({
            "x_r": np.ascontiguousarray(x[r * S_LOC:(r + 1) * S_LOC, :]),
            "lng": lng_b, "lnb": lnb_b, "b2b": b2_b, "b1t": b1_t,
            "wqkv": np.ascontiguousarray(wqkv_r),
            "wv": wv_blk,
            "wo": wo_b, "w1": w1_b, "w2": w2_b,
            "msk": msk, "onc": onc,
        })
    return in_maps


def kernel(**inputs) -> np.ndarray:
    nc = _build()
    in_maps = _host_prep(inputs)
    res = run_bass_kernel_spmd(
        nc, in_maps, core_ids=list(range(N_CORES)), trace=TRACE)
    _CACHE["last_result"] = res
    out = np.concatenate([res.results[r]["out_r"] for r in range(N_CORES)], axis=0)
    return out.reshape(1, S, D)
